# revision 1
# baseline (speedup 1.0000x reference)
"""Cayley soliton propagator — Trainium2 Bass kernel.

Math: the reference runs 20 non-converging PCG iterations on
(I + i*k*H) x = (I - i*k*H) rot(psi) per (batch,token) system, where H is a
fixed circulant stencil along D and the preconditioner is a constant complex
scalar.  Every CG iterate therefore lives in the Krylov space of H applied to
b, so per system the output is x = F^-1( G . F b ) with a per-(frequency,
system) complex gain G determined solely by the Chebyshev-weighted power
spectrum moments nu_n[s] = sum_f T_n(mu_f) |b_hat[f,s]|^2.

Device kernel 1: elementwise phase rotation -> rot, forward modified-DFT
matmul b_hat = (1 - i*k*lam) . F rot (PE), power spectrum, moment matmul.
Host: 20-iteration CG recurrence in Chebyshev coefficient space (fp64,
vectorized over all systems) -> 21 complex gain coefficients per system.
Device kernel 2: evaluate G from coefficients (PE), x_hat = G . b_hat,
inverse-DFT matmul (PE), interleave to [..., 2] and store.

Sharding: data-parallel over the flattened system axis N=B*S across 8 cores.
"""

import sys

for _p in ("/opt/trn_rl_repo",):
    if _p not in sys.path:
        sys.path.insert(0, _p)

import numpy as np
import concourse.bass as bass
import concourse.tile as tile
from concourse import bacc, mybir
from concourse.bass_utils import run_bass_kernel_spmd
from concourse.masks import make_identity

f32 = mybir.dt.float32
OP = mybir.AluOpType
AF = mybir.ActivationFunctionType

# ---- problem constants (hardcoded per contract) ----
B, S, D = 4, 4096, 512
N = B * S                       # 16384 systems
NCORES = 8
NSYS = N // NCORES              # 2048 systems per core
NTILE = NSYS // 128             # 16 sys-tiles of 128 per core
NSUP = NSYS // 512              # 4 supers of 512 systems per core
DT = 0.1
KAP = DT / 2.0                  # 0.05
NIT = 20
NCOEF = NIT + 1                 # 21
NMOM = 2 * NIT + 1              # 41
NUM_SCALES, BASE_SPARSITY = 3, 5
OFFSETS = [(2 ** s) * j for s in range(NUM_SCALES) for j in range(1, BASE_SPARSITY + 1)]
KCH = 4                         # 512/128 chunks


def _host_matrices(edge_weights, alpha):
    """All constant matrices, fp64 -> f32."""
    w = edge_weights.reshape(-1).astype(np.float64)
    f = np.arange(D)
    deg = 2.0 * w.sum()
    lam = deg - sum(w[k] * 2.0 * np.cos(2 * np.pi * OFFSETS[k] * f / D)
                    for k in range(len(w)))
    lmax = lam.max()
    mu = 2.0 * lam / lmax - 1.0

    T = np.zeros((NMOM, D))
    T[0] = 1.0
    T[1] = mu
    for n in range(2, NMOM):
        T[n] = 2 * mu * T[n - 1] - T[n - 2]

    dmat = np.outer(f, f)
    F = np.exp(-2j * np.pi * dmat / D)            # F[f, d]
    Fp = (1.0 - 1j * KAP * lam)[:, None] * F      # modified forward DFT
    # device fields: R = -(rot_r), I2 = rot_i  (sign of rot_r folded here)
    # bhat_r = Fp_r rot_r - Fp_i rot_i -> (-Fp_r) R + (-Fp_i) I2
    # bhat_i = Fp_i rot_r + Fp_r rot_i -> (-Fp_i) R + ( Fp_r) I2
    # device computes rot_r, rot_i directly (true signs):
    # bhat_r = Fp_r rot_r - Fp_i rot_i  -> A1 @ rot_rT + A2 @ rot_iT
    # bhat_i = Fp_i rot_r + Fp_r rot_i  -> A3 @ rot_rT + A1 @ rot_iT
    A1 = np.ascontiguousarray((Fp.real).T)        # lhsT layout [d, f]
    A2 = np.ascontiguousarray((-Fp.imag).T)
    A3 = np.ascontiguousarray((Fp.imag).T)
    Finv = np.exp(2j * np.pi * dmat / D) / D      # Finv[f, d]
    Fir = np.ascontiguousarray(Finv.real)
    Fii = np.ascontiguousarray(Finv.imag)
    Fin = np.ascontiguousarray(-Finv.imag)
    Tmomt = np.ascontiguousarray(T.T)             # [f, n] lhsT for moments
    Tgain = np.ascontiguousarray(T[:NCOEF])       # [n, f] lhsT for gain eval
    aabs = np.abs(alpha.astype(np.float64)).reshape(1, D)
    c = dict(A1=A1, A2=A2, A3=A3, Fir=Fir, Fii=Fii, Fin=Fin,
             Tmomt=Tmomt, Tgain=Tgain, aabs=aabs)
    c = {k: v.astype(np.float32) for k, v in c.items()}
    c["lam"] = lam
    c["lmax"] = lmax
    c["deg"] = deg
    return c


# ---------------------------------------------------------------- kernel 1
def _build_k1():
    nc = bacc.Bacc()
    pr_d = nc.declare_dram_parameter("pr", [NSYS, D], f32, isOutput=False)
    pi_d = nc.declare_dram_parameter("pi", [NSYS, D], f32, isOutput=False)
    A1_d = nc.declare_dram_parameter("A1", [D, D], f32, isOutput=False)
    A2_d = nc.declare_dram_parameter("A2", [D, D], f32, isOutput=False)
    A3_d = nc.declare_dram_parameter("A3", [D, D], f32, isOutput=False)
    Tm_d = nc.declare_dram_parameter("Tmomt", [D, NMOM], f32, isOutput=False)
    aa_d = nc.declare_dram_parameter("aabs", [1, D], f32, isOutput=False)
    bhr_d = nc.declare_dram_parameter("bhr", [D, NSYS], f32, isOutput=True)
    bhi_d = nc.declare_dram_parameter("bhi", [D, NSYS], f32, isOutput=True)
    nu_d = nc.declare_dram_parameter("nu", [NMOM, NSYS], f32, isOutput=True)

    with tile.TileContext(nc) as tc:
        with tc.tile_pool(name="singles", bufs=1) as singles, \
             tc.tile_pool(name="io", bufs=3) as io, \
             tc.tile_pool(name="tmp", bufs=2) as tmp, \
             tc.tile_pool(name="cols", bufs=3) as colsp, \
             tc.tile_pool(name="rotT", bufs=2) as rotTp, \
             tc.tile_pool(name="bh", bufs=2) as bhp, \
             tc.tile_pool(name="pst", bufs=2, space="PSUM") as pst, \
             tc.tile_pool(name="psb", bufs=2, space="PSUM") as psb, \
             tc.tile_pool(name="psn", bufs=1, space="PSUM") as psn:

            # constants
            A1_s = singles.tile([128, KCH * D], f32)   # chunk k at cols k*512
            A2_s = singles.tile([128, KCH * D], f32)
            A3_s = singles.tile([128, KCH * D], f32)
            for k in range(KCH):
                nc.sync.dma_start(A1_s[:, k * D:(k + 1) * D], A1_d[k * 128:(k + 1) * 128, :])
                nc.sync.dma_start(A2_s[:, k * D:(k + 1) * D], A2_d[k * 128:(k + 1) * 128, :])
                nc.sync.dma_start(A3_s[:, k * D:(k + 1) * D], A3_d[k * 128:(k + 1) * 128, :])
            Tm_s = singles.tile([128, KCH * NMOM], f32)
            for k in range(KCH):
                nc.sync.dma_start(Tm_s[:, k * NMOM:(k + 1) * NMOM], Tm_d[k * 128:(k + 1) * 128, :])
            aab = singles.tile([128, D], f32)
            nc.gpsimd.dma_start(out=aab[:], in_=aa_d[:].to_broadcast([128, D]))
            ident = singles.tile([128, 128], f32)
            make_identity(nc, ident[:])
            nhalfpi = singles.tile([128, 1], f32)
            nc.vector.memset(nhalfpi[:], float(-np.pi / 2))

            for sup in range(NSUP):
                rrT = [rotTp.tile([128, 512], f32, name=f"rrT{k}", tag=f"rrT{k}") for k in range(KCH)]
                riT = [rotTp.tile([128, 512], f32, name=f"riT{k}", tag=f"riT{k}") for k in range(KCH)]
                for j in range(4):          # 4 sys-tiles of 128 in this super
                    t0 = sup * 4 + j
                    rows = slice(t0 * 128, (t0 + 1) * 128)
                    prt = io.tile([128, D], f32, tag="prt")
                    pit = io.tile([128, D], f32, tag="pit")
                    nc.sync.dma_start(prt[:], pr_d[rows, :])
                    nc.sync.dma_start(pit[:], pi_d[rows, :])

                    cols = colsp.tile([128, 16], f32, tag="cols")
                    sqr = tmp.tile([128, D], f32, tag="ta")
                    sqi = tmp.tile([128, D], f32, tag="tb")
                    nc.vector.scalar_tensor_tensor(
                        out=sqr[:], in0=prt[:], scalar=1.0, in1=prt[:],
                        op0=OP.mult, op1=OP.mult, accum_out=cols[:, 0:1])
                    nc.vector.scalar_tensor_tensor(
                        out=sqi[:], in0=pit[:], scalar=1.0, in1=pit[:],
                        op0=OP.mult, op1=OP.mult, accum_out=cols[:, 1:2])
                    ir = tmp.tile([128, D], f32, tag="ir")
                    nc.gpsimd.tensor_tensor(out=ir[:], in0=sqr[:], in1=sqi[:], op=OP.add)
                    # norm_in = c0+c1 ; rm = 1/max(norm_in/512, 1e-6) ; nrm = -rm
                    nc.vector.tensor_tensor(out=cols[:, 2:3], in0=cols[:, 0:1],
                                            in1=cols[:, 1:2], op=OP.add)
                    nc.vector.tensor_scalar(out=cols[:, 3:4], in0=cols[:, 2:3],
                                            scalar1=1.0 / D, scalar2=1e-6,
                                            op0=OP.mult, op1=OP.max)
                    nc.vector.reciprocal(out=cols[:, 4:5], in_=cols[:, 3:4])
                    nc.vector.tensor_scalar(out=cols[:, 5:6], in0=cols[:, 4:5],
                                            scalar1=-1.0, scalar2=None, op0=OP.mult)
                    # u = exp(-ir*rm) in (0,1].  phase = pi - 2*pi*u, so with
                    # half-angle vars (ACT Sin args stay within [-pi, pi]):
                    #   shalf = sin(pi*u - pi/2), chalf = sin(pi*u)
                    #   cos_p = 1 - 2*shalf^2 ; sin_p = -2*shalf*chalf
                    u = tmp.tile([128, D], f32, tag="u")
                    nc.scalar.activation(out=u[:], in_=ir[:], func=AF.Exp,
                                         bias=0.0, scale=cols[:, 5:6])
                    shalf = tmp.tile([128, D], f32, tag="ta")
                    nc.scalar.activation(out=shalf[:], in_=u[:], func=AF.Sin,
                                         bias=nhalfpi[:], scale=float(np.pi))
                    chalf = tmp.tile([128, D], f32, tag="tb")
                    nc.scalar.activation(out=chalf[:], in_=u[:], func=AF.Sin,
                                         bias=0.0, scale=float(np.pi))
                    q1 = tmp.tile([128, D], f32, tag="tm1")
                    nc.vector.tensor_tensor(out=q1[:], in0=shalf[:], in1=shalf[:], op=OP.mult)
                    cp = tmp.tile([128, D], f32, tag="cp")
                    nc.vector.tensor_scalar(out=cp[:], in0=q1[:], scalar1=-2.0,
                                            scalar2=1.0, op0=OP.mult, op1=OP.add)
                    q2 = tmp.tile([128, D], f32, tag="tm2")
                    nc.gpsimd.tensor_tensor(out=q2[:], in0=shalf[:], in1=chalf[:], op=OP.mult)
                    sp = tmp.tile([128, D], f32, tag="sp")
                    nc.vector.tensor_scalar(out=sp[:], in0=q2[:], scalar1=-2.0,
                                            scalar2=None, op0=OP.mult)
                    # env = min(1 + aabs*(ir*rm)^2, 10) ; renv = 1/env
                    tsq = tmp.tile([128, D], f32, tag="tsq")
                    nc.scalar.activation(out=tsq[:], in_=ir[:], func=AF.Square,
                                         bias=0.0, scale=cols[:, 4:5])
                    env = tmp.tile([128, D], f32, tag="env")
                    nc.vector.scalar_tensor_tensor(
                        out=env[:], in0=tsq[:], scalar=1.0, in1=aab[:],
                        op0=OP.mult, op1=OP.mult)
                    nc.vector.tensor_scalar(out=env[:], in0=env[:],
                                            scalar1=1.0, scalar2=10.0,
                                            op0=OP.add, op1=OP.min)
                    renv = tmp.tile([128, D], f32, tag="renv")
                    nc.vector.reciprocal_approx_fast(out=renv[:], in_=env[:])
                    renv2 = tmp.tile([128, D], f32, tag="renv2")
                    nc.scalar.activation(out=renv2[:], in_=renv[:], func=AF.Square)
                    # norm_rot = sum(ir * renv^2) (exact identity: |rot|^2 = ir pointwise)
                    scr = tmp.tile([128, D], f32, tag="tsq")
                    nc.vector.scalar_tensor_tensor(
                        out=scr[:], in0=ir[:], scalar=1.0, in1=renv2[:],
                        op0=OP.mult, op1=OP.mult, accum_out=cols[:, 6:7])
                    # sc = min(sqrt((ni+1e-8)/(nr+1e-8)), 10)
                    nc.vector.tensor_scalar(out=cols[:, 7:8], in0=cols[:, 6:7],
                                            scalar1=1e-8, scalar2=None, op0=OP.add)
                    nc.vector.reciprocal(out=cols[:, 8:9], in_=cols[:, 7:8])
                    nc.vector.tensor_scalar(out=cols[:, 9:10], in0=cols[:, 2:3],
                                            scalar1=1e-8, scalar2=None, op0=OP.add)
                    nc.vector.tensor_tensor(out=cols[:, 10:11], in0=cols[:, 8:9],
                                            in1=cols[:, 9:10], op=OP.mult)
                    nc.scalar.activation(out=cols[:, 11:12], in_=cols[:, 10:11], func=AF.Sqrt)
                    nc.vector.tensor_scalar(out=cols[:, 12:13], in0=cols[:, 11:12],
                                            scalar1=10.0, scalar2=None, op0=OP.min)
                    # fac = renv * sc ; R = pr*c2t + pi*s2t ; I2 = pr*s2t - pi*c2t
                    fac = tmp.tile([128, D], f32, tag="fac")
                    nc.vector.tensor_scalar(out=fac[:], in0=renv[:],
                                            scalar1=cols[:, 12:13], scalar2=None,
                                            op0=OP.mult)
                    # rot_r = pr*cos_p - pi*sin_p ; rot_i = pr*sin_p + pi*cos_p
                    t1 = tmp.tile([128, D], f32, tag="tm1")
                    t2 = tmp.tile([128, D], f32, tag="tm2")
                    nc.vector.tensor_tensor(out=t1[:], in0=prt[:], in1=cp[:], op=OP.mult)
                    nc.gpsimd.tensor_tensor(out=t2[:], in0=pit[:], in1=sp[:], op=OP.mult)
                    Rt = tmp.tile([128, D], f32, tag="Rt")
                    nc.vector.tensor_tensor(out=Rt[:], in0=t1[:], in1=t2[:], op=OP.subtract)
                    t3 = tmp.tile([128, D], f32, tag="tm3")
                    t4 = tmp.tile([128, D], f32, tag="tm4")
                    nc.gpsimd.tensor_tensor(out=t3[:], in0=prt[:], in1=sp[:], op=OP.mult)
                    nc.vector.tensor_tensor(out=t4[:], in0=pit[:], in1=cp[:], op=OP.mult)
                    I2t = tmp.tile([128, D], f32, tag="I2t")
                    nc.vector.tensor_tensor(out=I2t[:], in0=t3[:], in1=t4[:], op=OP.add)
                    rr = tmp.tile([128, D], f32, tag="rr")
                    nc.vector.tensor_tensor(out=rr[:], in0=Rt[:], in1=fac[:], op=OP.mult)
                    ri = tmp.tile([128, D], f32, tag="ri")
                    nc.gpsimd.tensor_tensor(out=ri[:], in0=I2t[:], in1=fac[:], op=OP.mult)
                    # transpose into rrT/riT chunk tiles
                    for k in range(KCH):
                        pt = pst.tile([128, 128], f32, tag="pt")
                        nc.tensor.transpose(pt[:], rr[:, k * 128:(k + 1) * 128], ident[:])
                        nc.scalar.copy(rrT[k][:, j * 128:(j + 1) * 128], pt[:])
                        pt2 = pst.tile([128, 128], f32, tag="pt")
                        nc.tensor.transpose(pt2[:], ri[:, k * 128:(k + 1) * 128], ident[:])
                        nc.scalar.copy(riT[k][:, j * 128:(j + 1) * 128], pt2[:])

                # forward matmul: bhat chunks for this super
                scols = slice(sup * 512, (sup + 1) * 512)
                bhr_s = [bhp.tile([128, 512], f32, name=f"bhrs{m}", tag=f"bhr{m}") for m in range(KCH)]
                bhi_s = [bhp.tile([128, 512], f32, name=f"bhis{m}", tag=f"bhi{m}") for m in range(KCH)]
                pi_ch = [bhp.tile([128, 512], f32, name=f"pich{m}", tag=f"pich{m}") for m in range(KCH)]
                for m in range(KCH):
                    pbr = psb.tile([128, 512], f32, tag="pbr")
                    for k in range(KCH):
                        nc.tensor.matmul(pbr[:], A1_s[:, k * D + m * 128: k * D + (m + 1) * 128],
                                         rrT[k][:], start=(k == 0), stop=False)
                    for k in range(KCH):
                        nc.tensor.matmul(pbr[:], A2_s[:, k * D + m * 128: k * D + (m + 1) * 128],
                                         riT[k][:], start=False, stop=(k == KCH - 1))
                    nc.scalar.copy(bhr_s[m][:], pbr[:])
                    pbi = psb.tile([128, 512], f32, tag="pbi")
                    for k in range(KCH):
                        nc.tensor.matmul(pbi[:], A3_s[:, k * D + m * 128: k * D + (m + 1) * 128],
                                         rrT[k][:], start=(k == 0), stop=False)
                    for k in range(KCH):
                        nc.tensor.matmul(pbi[:], A1_s[:, k * D + m * 128: k * D + (m + 1) * 128],
                                         riT[k][:], start=False, stop=(k == KCH - 1))
                    nc.scalar.copy(bhi_s[m][:], pbi[:])
                    nc.sync.dma_start(bhr_d[m * 128:(m + 1) * 128, scols], bhr_s[m][:])
                    nc.sync.dma_start(bhi_d[m * 128:(m + 1) * 128, scols], bhi_s[m][:])
                    # power spectrum chunk
                    p1 = tmp.tile([128, 512], f32, tag="tm1")
                    nc.scalar.activation(out=p1[:], in_=bhr_s[m][:], func=AF.Square)
                    p2 = tmp.tile([128, 512], f32, tag="tm2")
                    nc.vector.tensor_tensor(out=p2[:], in0=bhi_s[m][:], in1=bhi_s[m][:],
                                            op=OP.mult)
                    nc.gpsimd.tensor_tensor(out=pi_ch[m][:], in0=p1[:], in1=p2[:], op=OP.add)
                # moments
                pnu = psn.tile([NMOM, 512], f32, tag="pnu")
                for k in range(KCH):
                    nc.tensor.matmul(pnu[:], Tm_s[:, k * NMOM:(k + 1) * NMOM],
                                     pi_ch[k][:], start=(k == 0), stop=(k == KCH - 1))
                nu_s = bhp.tile([NMOM, 512], f32, tag="nus")
                nc.scalar.copy(nu_s[:], pnu[:])
                nc.sync.dma_start(nu_d[:, scols], nu_s[:])
    nc.compile()
    return nc


# ---------------------------------------------------------------- kernel 2
def _build_k2():
    nc = bacc.Bacc()
    bhr_d = nc.declare_dram_parameter("bhr", [D, NSYS], f32, isOutput=False)
    bhi_d = nc.declare_dram_parameter("bhi", [D, NSYS], f32, isOutput=False)
    gr_d = nc.declare_dram_parameter("gr", [NCOEF, NSYS], f32, isOutput=False)
    gi_d = nc.declare_dram_parameter("gi", [NCOEF, NSYS], f32, isOutput=False)
    Fir_d = nc.declare_dram_parameter("Fir", [D, D], f32, isOutput=False)
    Fii_d = nc.declare_dram_parameter("Fii", [D, D], f32, isOutput=False)
    Fin_d = nc.declare_dram_parameter("Fin", [D, D], f32, isOutput=False)
    Tg_d = nc.declare_dram_parameter("Tgain", [NCOEF, D], f32, isOutput=False)
    x_d = nc.declare_dram_parameter("xout", [NSYS, 2 * D], f32, isOutput=True)

    with tile.TileContext(nc) as tc:
        with tc.tile_pool(name="singles", bufs=1) as singles, \
             tc.tile_pool(name="io", bufs=2) as io, \
             tc.tile_pool(name="tmp", bufs=2) as tmp, \
             tc.tile_pool(name="outp", bufs=3) as outp, \
             tc.tile_pool(name="psg", bufs=2, space="PSUM") as psg, \
             tc.tile_pool(name="psx", bufs=2, space="PSUM") as psx:

            Fir_s = singles.tile([128, KCH * D], f32)
            Fii_s = singles.tile([128, KCH * D], f32)
            Fin_s = singles.tile([128, KCH * D], f32)
            for k in range(KCH):
                nc.sync.dma_start(Fir_s[:, k * D:(k + 1) * D], Fir_d[k * 128:(k + 1) * 128, :])
                nc.sync.dma_start(Fii_s[:, k * D:(k + 1) * D], Fii_d[k * 128:(k + 1) * 128, :])
                nc.sync.dma_start(Fin_s[:, k * D:(k + 1) * D], Fin_d[k * 128:(k + 1) * 128, :])
            Tg_s = singles.tile([NCOEF, D], f32)
            nc.sync.dma_start(Tg_s[:], Tg_d[:])

            for sup in range(NSUP):
                scols = slice(sup * 512, (sup + 1) * 512)
                bhr_s = [io.tile([128, 512], f32, name=f"bhrs{k}", tag=f"bhr{k}") for k in range(KCH)]
                bhi_s = [io.tile([128, 512], f32, name=f"bhis{k}", tag=f"bhi{k}") for k in range(KCH)]
                for k in range(KCH):
                    nc.sync.dma_start(bhr_s[k][:], bhr_d[k * 128:(k + 1) * 128, scols])
                    nc.sync.dma_start(bhi_s[k][:], bhi_d[k * 128:(k + 1) * 128, scols])
                grt = io.tile([NCOEF, 512], f32, tag="grt")
                git = io.tile([NCOEF, 512], f32, tag="git")
                nc.sync.dma_start(grt[:], gr_d[:, scols])
                nc.sync.dma_start(git[:], gi_d[:, scols])

                xhr = [tmp.tile([128, 512], f32, name=f"xhr{m}", tag=f"xhr{m}") for m in range(KCH)]
                xhi = [tmp.tile([128, 512], f32, name=f"xhi{m}", tag=f"xhi{m}") for m in range(KCH)]
                for m in range(KCH):
                    pgr = psg.tile([128, 512], f32, tag="pgr")
                    nc.tensor.matmul(pgr[:], Tg_s[:, m * 128:(m + 1) * 128], grt[:],
                                     start=True, stop=True)
                    pgi = psg.tile([128, 512], f32, tag="pgi")
                    nc.tensor.matmul(pgi[:], Tg_s[:, m * 128:(m + 1) * 128], git[:],
                                     start=True, stop=True)
                    Gr = tmp.tile([128, 512], f32, tag="Gr")
                    nc.scalar.copy(Gr[:], pgr[:])
                    Gi = tmp.tile([128, 512], f32, tag="Gi")
                    nc.scalar.copy(Gi[:], pgi[:])
                    a = tmp.tile([128, 512], f32, tag="xa")
                    b = tmp.tile([128, 512], f32, tag="xb")
                    nc.vector.tensor_tensor(out=a[:], in0=Gr[:], in1=bhr_s[m][:], op=OP.mult)
                    nc.gpsimd.tensor_tensor(out=b[:], in0=Gi[:], in1=bhi_s[m][:], op=OP.mult)
                    nc.vector.tensor_tensor(out=xhr[m][:], in0=a[:], in1=b[:], op=OP.subtract)
                    a2 = tmp.tile([128, 512], f32, tag="xa2")
                    b2 = tmp.tile([128, 512], f32, tag="xb2")
                    nc.gpsimd.tensor_tensor(out=a2[:], in0=Gi[:], in1=bhr_s[m][:], op=OP.mult)
                    nc.vector.tensor_tensor(out=b2[:], in0=Gr[:], in1=bhi_s[m][:], op=OP.mult)
                    nc.vector.tensor_tensor(out=xhi[m][:], in0=a2[:], in1=b2[:], op=OP.add)

                for j in range(4):
                    t0 = sup * 4 + j
                    jcols = slice(j * 128, (j + 1) * 128)
                    pxr = psx.tile([128, 512], f32, tag="pxr")
                    for k in range(KCH):
                        nc.tensor.matmul(pxr[:], xhr[k][:, jcols], Fir_s[:, k * D:(k + 1) * D],
                                         start=(k == 0), stop=False)
                    for k in range(KCH):
                        nc.tensor.matmul(pxr[:], xhi[k][:, jcols], Fin_s[:, k * D:(k + 1) * D],
                                         start=False, stop=(k == KCH - 1))
                    pxi = psx.tile([128, 512], f32, tag="pxi")
                    for k in range(KCH):
                        nc.tensor.matmul(pxi[:], xhr[k][:, jcols], Fii_s[:, k * D:(k + 1) * D],
                                         start=(k == 0), stop=False)
                    for k in range(KCH):
                        nc.tensor.matmul(pxi[:], xhi[k][:, jcols], Fir_s[:, k * D:(k + 1) * D],
                                         start=False, stop=(k == KCH - 1))
                    ot = outp.tile([128, 2 * D], f32, tag="ot")
                    ov = ot[:].rearrange("p (d t) -> p d t", t=2)
                    nc.scalar.copy(ov[:, :, 0], pxr[:])
                    nc.vector.tensor_copy(ov[:, :, 1], pxi[:])
                    nc.sync.dma_start(x_d[t0 * 128:(t0 + 1) * 128, :], ot[:])
    nc.compile()
    return nc


# ------------------------------------------------------------- host solver
def _host_recurrence(nu, lmax, deg):
    """nu: [Nsys, NMOM] f32 device moments -> gain coefficients [Nsys, NCOEF] c128."""
    nu = nu.astype(np.float64)
    n = nu.shape[0]
    ia = np.arange(NCOEF)
    Aidx, Bidx = np.meshgrid(ia, ia, indexing="ij")
    W = (0.5 / D) * (nu[:, Aidx + Bidx] + nu[:, np.abs(Aidx - Bidx)])

    c1 = lmax / 2.0
    c0 = lmax / 2.0

    def mulH(c):
        o = c0 * c
        t = np.zeros_like(c)
        t[:, 1] += c[:, 0]
        t[:, 0] += 0.5 * c[:, 1]
        t[:, 2:] += 0.5 * c[:, 1:-1]
        t[:, 1:-1] += 0.5 * c[:, 2:]
        return o + c1 * t

    dk = KAP * deg
    inv_s2 = 1.0 / (1.0 + dk * dk)
    xr = np.zeros((n, NCOEF))
    xi = np.zeros((n, NCOEF))
    rr = np.zeros((n, NCOEF))
    rr[:, 0] = 1.0
    ri = np.zeros((n, NCOEF))
    qr = rr.copy()
    qi = dk * rr                      # q0 = (1 + i*dk) r0
    rnorm = W[:, 0, 0].astype(np.float64)     # <r0, r0>
    rz = inv_s2 * rnorm
    for _ in range(NIT):
        Hqr = mulH(qr)
        Hqi = mulH(qi)
        aqr = qr - KAP * Hqi          # aq = q + i*kap*H q
        aqi = qi + KAP * Hqr
        # single W pass per iteration; <r',r'> expanded to avoid a second
        WA = np.matmul(W, np.stack([aqr, aqi], -1))
        War, Wai = WA[..., 0], WA[..., 1]
        q_aq = (qr * War).sum(1) + (qi * Wai).sum(1)
        aq_aq = (aqr * War).sum(1) + (aqi * Wai).sum(1)
        r_aq = (rr * War).sum(1) + (ri * Wai).sum(1)
        pAp = inv_s2 * inv_s2 * q_aq
        a = rz / (pAp + 1e-30)
        ai = a * inv_s2
        xr += ai[:, None] * qr
        xi += ai[:, None] * qi
        rr = rr - ai[:, None] * aqr
        ri = ri - ai[:, None] * aqi
        rnorm = rnorm - 2 * ai * r_aq + ai * ai * aq_aq
        rz_new = inv_s2 * rnorm
        beta = (rz_new / (rz + 1e-30))[:, None]
        qr = rr - dk * ri + beta * qr   # q' = (1 + i*dk) r' + beta*q
        qi = dk * rr + ri + beta * qi
        rz = rz_new
    return xr + 1j * xi


_cache = {}


def _make_exec(nc, replicated=()):
    """Mirror of bass2jax.run_bass_via_pjrt's multi-core path, but returning
    the jitted callable so outputs can stay device-resident between kernels.
    Inputs/outputs are GLOBAL arrays (axis 0 = concat of per-core shards)."""
    import jax
    from jax.sharding import Mesh, PartitionSpec
    from jax.experimental.shard_map import shard_map
    from concourse import bass2jax, mybir as _mb

    bass2jax.install_neuronx_cc_hook()
    partition_name = (nc.partition_id_tensor.name
                      if nc.partition_id_tensor else None)
    in_names, out_names, out_avals, zero_outs = [], [], [], []
    for alloc in nc.m.functions[0].allocations:
        if not isinstance(alloc, _mb.MemoryLocationSet):
            continue
        name = alloc.memorylocations[0].name
        if alloc.kind == "ExternalInput":
            if name != partition_name:
                in_names.append(name)
        elif alloc.kind == "ExternalOutput":
            out_names.append(name)
            shape = tuple(alloc.tensor_shape)
            dtype = _mb.dt.np(alloc.dtype)
            out_avals.append(jax.core.ShapedArray(shape, dtype))
            zero_outs.append(((NCORES * shape[0],) + shape[1:], dtype))
    n_params = len(in_names)
    all_in = list(in_names) + list(out_names)
    if partition_name is not None:
        all_in.append(partition_name)

    def _body(*args):
        operands = list(args)
        if partition_name is not None:
            operands.append(bass2jax.partition_id_tensor())
        return tuple(bass2jax._bass_exec_p.bind(
            *operands,
            out_avals=tuple(out_avals),
            in_names=tuple(all_in),
            out_names=tuple(out_names),
            lowering_input_output_aliases=(),
            sim_require_finite=True,
            sim_require_nnan=True,
            nc=nc,
        ))

    devices = jax.devices()[:NCORES]
    mesh = Mesh(np.asarray(devices), ("core",))
    n_outs = len(out_names)
    in_specs = tuple(
        PartitionSpec() if nm in replicated else PartitionSpec("core")
        for nm in in_names
    ) + (PartitionSpec("core"),) * n_outs
    sharded = jax.jit(
        shard_map(_body, mesh=mesh,
                  in_specs=in_specs,
                  out_specs=(PartitionSpec("core"),) * n_outs,
                  check_rep=False),
        donate_argnums=tuple(range(n_params, n_params + n_outs)),
        keep_unused=True,
    )

    def run(feed):  # feed: dict name -> global array (np or jax)
        import jax.numpy as jnp
        args = [feed[n] for n in in_names]
        # donated output buffers created device-side (no host->device upload)
        zs = [jnp.zeros(shp, dt) for shp, dt in zero_outs]
        return sharded(*args, *zs)

    return run, out_names


def kernel(psi_r, psi_i, alpha, edge_weights, trace=False):
    psi_r = np.ascontiguousarray(np.asarray(psi_r, np.float32).reshape(N, D))
    psi_i = np.ascontiguousarray(np.asarray(psi_i, np.float32).reshape(N, D))
    c = _host_matrices(np.asarray(edge_weights, np.float64),
                       np.asarray(alpha, np.float64))
    try:
        return _kernel_chained(psi_r, psi_i, c, trace)
    except Exception:
        return _kernel_safe(psi_r, psi_i, c, trace)


def _solve_gain(nu_all, c):
    """nu_all [NMOM, N] -> gain coefficient planes gr, gi [NCOEF, N] f32."""
    xc = _host_recurrence(nu_all.T, c["lmax"], c["deg"])
    gr = np.ascontiguousarray(xc.real.T.astype(np.float32))
    gi = np.ascontiguousarray(xc.imag.T.astype(np.float32))
    return gr, gi


def _kernel_chained(psi_r, psi_i, c, trace):
    """Fast path: bhat stays device-resident between the two kernels."""
    import time
    if "k1" not in _cache:
        _cache["k1"] = _build_k1()
        _cache["k2"] = _build_k2()
    if "ex1" not in _cache:
        _cache["ex1"] = _make_exec(
            _cache["k1"], replicated=("A1", "A2", "A3", "Tmomt", "aabs"))
        _cache["ex2"] = _make_exec(
            _cache["k2"], replicated=("Fir", "Fii", "Fin", "Tgain"))
    run1, out1n = _cache["ex1"]
    run2, out2n = _cache["ex2"]

    feed1 = dict(pr=psi_r, pi=psi_i, A1=c["A1"], A2=c["A2"],
                 A3=c["A3"], Tmomt=c["Tmomt"], aabs=c["aabs"])
    t0 = time.perf_counter()
    o1 = dict(zip(out1n, run1(feed1)))
    nu_blk = np.asarray(o1["nu"]).reshape(NCORES, NMOM, NSYS)
    t1 = int((time.perf_counter() - t0) * 1e9)
    nu_all = np.concatenate(list(nu_blk), axis=1)          # [NMOM, N]

    gr, gi = _solve_gain(nu_all, c)
    gr_g = np.concatenate([gr[:, ci * NSYS:(ci + 1) * NSYS] for ci in range(NCORES)], axis=0)
    gi_g = np.concatenate([gi[:, ci * NSYS:(ci + 1) * NSYS] for ci in range(NCORES)], axis=0)

    feed2 = dict(bhr=o1["bhr"], bhi=o1["bhi"], gr=gr_g, gi=gi_g,
                 Fir=c["Fir"], Fii=c["Fii"], Fin=c["Fin"],
                 Tgain=c["Tgain"])
    t0 = time.perf_counter()
    o2 = dict(zip(out2n, run2(feed2)))
    x = np.asarray(o2["xout"])                              # [N, 2*D]
    t2 = int((time.perf_counter() - t0) * 1e9)
    out = x.reshape(B, S, D, 2)
    if trace:
        return out, (t1, t2)
    return out


def _kernel_safe(psi_r, psi_i, c, trace):
    if "k1" not in _cache:
        _cache["k1"] = _build_k1()
        _cache["k2"] = _build_k2()
    k1, k2 = _cache["k1"], _cache["k2"]

    core_ids = list(range(NCORES))
    in1 = []
    for ci in core_ids:
        rows = slice(ci * NSYS, (ci + 1) * NSYS)
        in1.append(dict(pr=psi_r[rows], pi=psi_i[rows],
                        A1=c["A1"], A2=c["A2"], A3=c["A3"],
                        Tmomt=c["Tmomt"], aabs=c["aabs"]))
    res1 = run_bass_kernel_spmd(k1, in1, core_ids)
    t1 = None
    if trace:
        import time
        t0 = time.perf_counter()
        res1 = run_bass_kernel_spmd(k1, in1, core_ids)
        t1 = int((time.perf_counter() - t0) * 1e9)

    nu = np.concatenate([res1.results[ci]["nu"] for ci in core_ids], axis=1)
    bhr = [res1.results[ci]["bhr"] for ci in core_ids]
    bhi = [res1.results[ci]["bhi"] for ci in core_ids]

    gr, gi = _solve_gain(nu, c)

    in2 = []
    for ci in core_ids:
        colsl = slice(ci * NSYS, (ci + 1) * NSYS)
        in2.append(dict(bhr=bhr[ci], bhi=bhi[ci],
                        gr=np.ascontiguousarray(gr[:, colsl]),
                        gi=np.ascontiguousarray(gi[:, colsl]),
                        Fir=c["Fir"], Fii=c["Fii"], Fin=c["Fin"],
                        Tgain=c["Tgain"]))
    res2 = run_bass_kernel_spmd(k2, in2, core_ids)
    t2 = None
    if trace:
        import time
        t0 = time.perf_counter()
        res2 = run_bass_kernel_spmd(k2, in2, core_ids)
        t2 = int((time.perf_counter() - t0) * 1e9)

    x = np.concatenate([res2.results[ci]["xout"] for ci in core_ids], axis=0)
    out = x.reshape(B, S, D, 2)
    if trace:
        return out, (t1, t2)
    return out



# revision 6
# speedup vs baseline: 3.1681x; 3.1681x over previous
"""Cayley soliton propagator — Trainium2 Bass kernel (fused, fp16 I/O).

Math: the reference runs 20 PCG iterations on (I + i*k*H) x = (I - i*k*H) rot(psi)
per (batch,token) system, where H is a fixed circulant stencil along D.  H is
diagonal in the DFT basis, so the whole pipeline runs on-device per system:
elementwise phase rotation -> forward modified DFT (PE matmul, b_hat in
[system, frequency] layout) -> 20 fixed CG iterations in frequency space where
A = diag(1 + i*k*lam_f) and M^-1 is a constant complex scalar (all CG dot
products are frequency-space dots; the 1/D Parseval factor cancels in every
ratio, and Re<p,Ap> = ||p||^2, Re<r,z> = inv_s2*||r||^2 exactly) -> gain-free
inverse DFT (PE matmul) -> interleaved [.., D, 2] fp16 output.

I/O is fp16 both ways (axon host<->device tunnel is the bottleneck at
~30-40 MB/s); constant matrices are uploaded once and cached on device.

Sharding: data-parallel over the flattened system axis N=B*S across 8 cores.
"""

import sys

for _p in ("/opt/trn_rl_repo",):
    if _p not in sys.path:
        sys.path.insert(0, _p)

import numpy as np
import concourse.bass as bass
import concourse.tile as tile
from concourse import bacc, mybir
from concourse.bass_utils import run_bass_kernel_spmd
from concourse.masks import make_identity

f32 = mybir.dt.float32
f16 = mybir.dt.float16
OP = mybir.AluOpType
AF = mybir.ActivationFunctionType

# ---- problem constants (hardcoded per contract) ----
B, S, D = 4, 4096, 512
N = B * S                       # 16384 systems
NCORES = 8
NSYS = N // NCORES              # 2048 systems per core
NTILE = NSYS // 128             # 16 sys-tiles of 128 per core
DT = 0.1
KAP = DT / 2.0                  # 0.05
NIT = 20
NUM_SCALES, BASE_SPARSITY = 3, 5
OFFSETS = [(2 ** s) * j for s in range(NUM_SCALES) for j in range(1, BASE_SPARSITY + 1)]
KCH = 4                         # 512/128 chunks


def _host_matrices(edge_weights, alpha):
    """All constant matrices, fp64 -> f32."""
    w = edge_weights.reshape(-1).astype(np.float64)
    f = np.arange(D)
    deg = 2.0 * w.sum()
    lam = deg - sum(w[k] * 2.0 * np.cos(2 * np.pi * OFFSETS[k] * f / D)
                    for k in range(len(w)))
    dmat = np.outer(f, f)
    F = np.exp(-2j * np.pi * dmat / D)            # F[f, d]
    Fp = (1.0 - 1j * KAP * lam)[:, None] * F      # modified forward DFT
    # b_sf[s, f] = rot_r A1 + rot_i A2 ; bhi analogous (A* in lhsT layout [d, f])
    A1 = np.ascontiguousarray((Fp.real).T)
    A2 = np.ascontiguousarray((-Fp.imag).T)
    A3 = np.ascontiguousarray((Fp.imag).T)
    Finv = np.exp(2j * np.pi * dmat / D) / D      # Finv[f, d]
    Fir = np.ascontiguousarray(Finv.real)
    Fii = np.ascontiguousarray(Finv.imag)
    Fin = np.ascontiguousarray(-Finv.imag)
    aabs = np.abs(alpha.astype(np.float64)).reshape(1, D)
    c = dict(A1=A1, A2=A2, A3=A3, Fir=Fir, Fii=Fii, Fin=Fin,
             aabs=aabs, lam=lam.reshape(1, D))
    c = {k: v.astype(np.float32) for k, v in c.items()}
    c["deg"] = deg
    return c


# ---------------------------------------------------------------- fused kernel
def _build_kernel(deg):
    dk = float(KAP * deg)
    inv_s2 = float(1.0 / (1.0 + dk * dk))

    nc = bacc.Bacc()
    pr_d = nc.declare_dram_parameter("pr", [NSYS, D], f16, isOutput=False)
    pi_d = nc.declare_dram_parameter("pi", [NSYS, D], f16, isOutput=False)
    A1_d = nc.declare_dram_parameter("A1", [D, D], f32, isOutput=False)
    A2_d = nc.declare_dram_parameter("A2", [D, D], f32, isOutput=False)
    A3_d = nc.declare_dram_parameter("A3", [D, D], f32, isOutput=False)
    Fir_d = nc.declare_dram_parameter("Fir", [D, D], f32, isOutput=False)
    Fii_d = nc.declare_dram_parameter("Fii", [D, D], f32, isOutput=False)
    Fin_d = nc.declare_dram_parameter("Fin", [D, D], f32, isOutput=False)
    aa_d = nc.declare_dram_parameter("aabs", [1, D], f32, isOutput=False)
    lam_d = nc.declare_dram_parameter("lam", [1, D], f32, isOutput=False)
    x_d = nc.declare_dram_parameter("xout", [NSYS, 2 * D], f16, isOutput=True)

    with tile.TileContext(nc) as tc:
        with tc.tile_pool(name="singles", bufs=1) as singles, \
             tc.tile_pool(name="io", bufs=2) as io, \
             tc.tile_pool(name="tmp", bufs=2) as tmp, \
             tc.tile_pool(name="cols", bufs=2) as colsp, \
             tc.tile_pool(name="rotT", bufs=2) as rotTp, \
             tc.tile_pool(name="cg", bufs=2) as cgp, \
             tc.tile_pool(name="outp", bufs=2) as outp, \
             tc.tile_pool(name="pst", bufs=2, space="PSUM") as pst, \
             tc.tile_pool(name="psb", bufs=1, space="PSUM") as psb, \
             tc.tile_pool(name="psx", bufs=1, space="PSUM") as psx:

            # constants: chunk k of each [D, D] matrix sits at cols [k*D:(k+1)*D]
            A1_s = singles.tile([128, KCH * D], f32)
            A2_s = singles.tile([128, KCH * D], f32)
            A3_s = singles.tile([128, KCH * D], f32)
            Fir_s = singles.tile([128, KCH * D], f32)
            Fii_s = singles.tile([128, KCH * D], f32)
            Fin_s = singles.tile([128, KCH * D], f32)
            for k in range(KCH):
                nc.sync.dma_start(A1_s[:, k * D:(k + 1) * D], A1_d[k * 128:(k + 1) * 128, :])
                nc.sync.dma_start(A2_s[:, k * D:(k + 1) * D], A2_d[k * 128:(k + 1) * 128, :])
                nc.sync.dma_start(A3_s[:, k * D:(k + 1) * D], A3_d[k * 128:(k + 1) * 128, :])
                nc.sync.dma_start(Fir_s[:, k * D:(k + 1) * D], Fir_d[k * 128:(k + 1) * 128, :])
                nc.sync.dma_start(Fii_s[:, k * D:(k + 1) * D], Fii_d[k * 128:(k + 1) * 128, :])
                nc.sync.dma_start(Fin_s[:, k * D:(k + 1) * D], Fin_d[k * 128:(k + 1) * 128, :])
            aab = singles.tile([128, D], f32)
            nc.gpsimd.dma_start(out=aab[:], in_=aa_d[:].to_broadcast([128, D]))
            lam128 = singles.tile([128, D], f32)
            nc.gpsimd.dma_start(out=lam128[:], in_=lam_d[:].to_broadcast([128, D]))
            ident = singles.tile([128, 128], f32)
            make_identity(nc, ident[:])
            nhalfpi = singles.tile([128, 1], f32)
            nc.vector.memset(nhalfpi[:], float(-np.pi / 2))

            for t0 in range(NTILE):
                rows = slice(t0 * 128, (t0 + 1) * 128)
                prt16 = io.tile([128, D], f16, tag="prt16")
                pit16 = io.tile([128, D], f16, tag="pit16")
                nc.sync.dma_start(prt16[:], pr_d[rows, :])
                nc.sync.dma_start(pit16[:], pi_d[rows, :])
                prt = io.tile([128, D], f32, tag="prt")
                pit = io.tile([128, D], f32, tag="pit")
                nc.scalar.copy(prt[:], prt16[:])
                nc.vector.tensor_copy(pit[:], pit16[:])

                # ---- phase rotation (same math as reference) ----
                cols = colsp.tile([128, 16], f32, tag="cols")
                sqr = tmp.tile([128, D], f32, tag="ta")
                sqi = tmp.tile([128, D], f32, tag="tb")
                nc.vector.scalar_tensor_tensor(
                    out=sqr[:], in0=prt[:], scalar=1.0, in1=prt[:],
                    op0=OP.mult, op1=OP.mult, accum_out=cols[:, 0:1])
                nc.vector.scalar_tensor_tensor(
                    out=sqi[:], in0=pit[:], scalar=1.0, in1=pit[:],
                    op0=OP.mult, op1=OP.mult, accum_out=cols[:, 1:2])
                ir = tmp.tile([128, D], f32, tag="ir")
                nc.gpsimd.tensor_tensor(out=ir[:], in0=sqr[:], in1=sqi[:], op=OP.add)
                nc.vector.tensor_tensor(out=cols[:, 2:3], in0=cols[:, 0:1],
                                        in1=cols[:, 1:2], op=OP.add)
                nc.vector.tensor_scalar(out=cols[:, 3:4], in0=cols[:, 2:3],
                                        scalar1=1.0 / D, scalar2=1e-6,
                                        op0=OP.mult, op1=OP.max)
                nc.vector.reciprocal(out=cols[:, 4:5], in_=cols[:, 3:4])
                nc.vector.tensor_scalar(out=cols[:, 5:6], in0=cols[:, 4:5],
                                        scalar1=-1.0, scalar2=None, op0=OP.mult)
                # u = exp(-ir*rm); half-angle: shalf = sin(pi*u - pi/2), chalf = sin(pi*u)
                u = tmp.tile([128, D], f32, tag="u")
                nc.scalar.activation(out=u[:], in_=ir[:], func=AF.Exp,
                                     bias=0.0, scale=cols[:, 5:6])
                shalf = tmp.tile([128, D], f32, tag="ta")
                nc.scalar.activation(out=shalf[:], in_=u[:], func=AF.Sin,
                                     bias=nhalfpi[:], scale=float(np.pi))
                chalf = tmp.tile([128, D], f32, tag="tb")
                nc.scalar.activation(out=chalf[:], in_=u[:], func=AF.Sin,
                                     bias=0.0, scale=float(np.pi))
                q1 = tmp.tile([128, D], f32, tag="tm1")
                nc.vector.tensor_tensor(out=q1[:], in0=shalf[:], in1=shalf[:], op=OP.mult)
                cp = tmp.tile([128, D], f32, tag="cp")
                nc.vector.tensor_scalar(out=cp[:], in0=q1[:], scalar1=-2.0,
                                        scalar2=1.0, op0=OP.mult, op1=OP.add)
                q2 = tmp.tile([128, D], f32, tag="tm2")
                nc.gpsimd.tensor_tensor(out=q2[:], in0=shalf[:], in1=chalf[:], op=OP.mult)
                sp = tmp.tile([128, D], f32, tag="sp")
                nc.vector.tensor_scalar(out=sp[:], in0=q2[:], scalar1=-2.0,
                                        scalar2=None, op0=OP.mult)
                tsq = tmp.tile([128, D], f32, tag="tsq")
                nc.scalar.activation(out=tsq[:], in_=ir[:], func=AF.Square,
                                     bias=0.0, scale=cols[:, 4:5])
                env = tmp.tile([128, D], f32, tag="env")
                nc.vector.scalar_tensor_tensor(
                    out=env[:], in0=tsq[:], scalar=1.0, in1=aab[:],
                    op0=OP.mult, op1=OP.mult)
                nc.vector.tensor_scalar(out=env[:], in0=env[:],
                                        scalar1=1.0, scalar2=10.0,
                                        op0=OP.add, op1=OP.min)
                renv = tmp.tile([128, D], f32, tag="renv")
                nc.vector.reciprocal_approx_fast(out=renv[:], in_=env[:])
                renv2 = tmp.tile([128, D], f32, tag="renv2")
                nc.scalar.activation(out=renv2[:], in_=renv[:], func=AF.Square)
                scr = tmp.tile([128, D], f32, tag="tsq")
                nc.vector.scalar_tensor_tensor(
                    out=scr[:], in0=ir[:], scalar=1.0, in1=renv2[:],
                    op0=OP.mult, op1=OP.mult, accum_out=cols[:, 6:7])
                nc.vector.tensor_scalar(out=cols[:, 7:8], in0=cols[:, 6:7],
                                        scalar1=1e-8, scalar2=None, op0=OP.add)
                nc.vector.reciprocal(out=cols[:, 8:9], in_=cols[:, 7:8])
                nc.vector.tensor_scalar(out=cols[:, 9:10], in0=cols[:, 2:3],
                                        scalar1=1e-8, scalar2=None, op0=OP.add)
                nc.vector.tensor_tensor(out=cols[:, 10:11], in0=cols[:, 8:9],
                                        in1=cols[:, 9:10], op=OP.mult)
                nc.scalar.activation(out=cols[:, 11:12], in_=cols[:, 10:11], func=AF.Sqrt)
                nc.vector.tensor_scalar(out=cols[:, 12:13], in0=cols[:, 11:12],
                                        scalar1=10.0, scalar2=None, op0=OP.min)
                fac = tmp.tile([128, D], f32, tag="fac")
                nc.vector.tensor_scalar(out=fac[:], in0=renv[:],
                                        scalar1=cols[:, 12:13], scalar2=None,
                                        op0=OP.mult)
                t1 = tmp.tile([128, D], f32, tag="tm1")
                t2 = tmp.tile([128, D], f32, tag="tm2")
                nc.vector.tensor_tensor(out=t1[:], in0=prt[:], in1=cp[:], op=OP.mult)
                nc.gpsimd.tensor_tensor(out=t2[:], in0=pit[:], in1=sp[:], op=OP.mult)
                Rt = tmp.tile([128, D], f32, tag="u")
                nc.vector.tensor_tensor(out=Rt[:], in0=t1[:], in1=t2[:], op=OP.subtract)
                t3 = tmp.tile([128, D], f32, tag="ta")
                t4 = tmp.tile([128, D], f32, tag="tb")
                nc.gpsimd.tensor_tensor(out=t3[:], in0=prt[:], in1=sp[:], op=OP.mult)
                nc.vector.tensor_tensor(out=t4[:], in0=pit[:], in1=cp[:], op=OP.mult)
                I2t = tmp.tile([128, D], f32, tag="ir")
                nc.vector.tensor_tensor(out=I2t[:], in0=t3[:], in1=t4[:], op=OP.add)
                rr = tmp.tile([128, D], f32, tag="rr")
                nc.vector.tensor_tensor(out=rr[:], in0=Rt[:], in1=fac[:], op=OP.mult)
                ri = tmp.tile([128, D], f32, tag="ri")
                nc.gpsimd.tensor_tensor(out=ri[:], in0=I2t[:], in1=fac[:], op=OP.mult)

                # ---- transpose rot into [d, sys] chunks for the forward DFT ----
                rrT = [rotTp.tile([128, 128], f32, name=f"rrT{k}", tag=f"rrT{k}") for k in range(KCH)]
                riT = [rotTp.tile([128, 128], f32, name=f"riT{k}", tag=f"riT{k}") for k in range(KCH)]
                for k in range(KCH):
                    pt = pst.tile([128, 128], f32, tag="pt")
                    nc.tensor.transpose(pt[:], rr[:, k * 128:(k + 1) * 128], ident[:])
                    nc.scalar.copy(rrT[k][:], pt[:])
                    pt2 = pst.tile([128, 128], f32, tag="pt")
                    nc.tensor.transpose(pt2[:], ri[:, k * 128:(k + 1) * 128], ident[:])
                    nc.scalar.copy(riT[k][:], pt2[:])

                # ---- forward DFT: b in [sys, f] layout; b is also r0 ----
                pbr = psb.tile([128, D], f32, tag="pbr")
                for k in range(KCH):
                    nc.tensor.matmul(pbr[:], rrT[k][:], A1_s[:, k * D:(k + 1) * D],
                                     start=(k == 0), stop=False)
                for k in range(KCH):
                    nc.tensor.matmul(pbr[:], riT[k][:], A2_s[:, k * D:(k + 1) * D],
                                     start=False, stop=(k == KCH - 1))
                pbi = psb.tile([128, D], f32, tag="pbi")
                for k in range(KCH):
                    nc.tensor.matmul(pbi[:], rrT[k][:], A3_s[:, k * D:(k + 1) * D],
                                     start=(k == 0), stop=False)
                for k in range(KCH):
                    nc.tensor.matmul(pbi[:], riT[k][:], A1_s[:, k * D:(k + 1) * D],
                                     start=False, stop=(k == KCH - 1))

                # ---- CG state ----
                rr_ = cgp.tile([128, D], f32, tag="rr_")
                ri_ = cgp.tile([128, D], f32, tag="ri_")
                nc.scalar.copy(rr_[:], pbr[:])
                nc.scalar.copy(ri_[:], pbi[:])
                xr = cgp.tile([128, D], f32, tag="xr")
                xi = cgp.tile([128, D], f32, tag="xi")
                nc.vector.memset(xr[:], 0.0)
                nc.vector.memset(xi[:], 0.0)
                pr_ = cgp.tile([128, D], f32, tag="pr_")
                pi_ = cgp.tile([128, D], f32, tag="pi_")
                # p0 = M^-1 r0
                nc.vector.scalar_tensor_tensor(
                    out=pr_[:], in0=ri_[:], scalar=-dk, in1=rr_[:],
                    op0=OP.mult, op1=OP.add)
                nc.vector.tensor_scalar(out=pr_[:], in0=pr_[:], scalar1=inv_s2,
                                        scalar2=None, op0=OP.mult)
                nc.vector.scalar_tensor_tensor(
                    out=pi_[:], in0=rr_[:], scalar=dk, in1=ri_[:],
                    op0=OP.mult, op1=OP.add)
                nc.vector.tensor_scalar(out=pi_[:], in0=pi_[:], scalar1=inv_s2,
                                        scalar2=None, op0=OP.mult)
                ccol = colsp.tile([128, 24], f32, tag="ccol")
                scr1 = cgp.tile([128, D], f32, tag="scr")
                scr2 = scr1
                tl1 = cgp.tile([128, D], f32, tag="tl1")
                tl2 = cgp.tile([128, D], f32, tag="tl2")
                # rz0 = inv_s2 * (||rr||^2 + ||ri||^2)
                nc.vector.scalar_tensor_tensor(
                    out=scr1[:], in0=rr_[:], scalar=1.0, in1=rr_[:],
                    op0=OP.mult, op1=OP.mult, accum_out=ccol[:, 0:1])
                nc.vector.scalar_tensor_tensor(
                    out=scr2[:], in0=ri_[:], scalar=1.0, in1=ri_[:],
                    op0=OP.mult, op1=OP.mult, accum_out=ccol[:, 1:2])
                nc.vector.tensor_tensor(out=ccol[:, 2:3], in0=ccol[:, 0:1],
                                        in1=ccol[:, 1:2], op=OP.add)
                RZ = ccol[:, 3:4]
                nc.vector.tensor_scalar(out=RZ, in0=ccol[:, 2:3],
                                        scalar1=inv_s2, scalar2=None, op0=OP.mult)

                cPA = ccol[:, 4:5]   # pAp
                cA = ccol[:, 5:6]
                cB = ccol[:, 6:7]
                cAcc = ccol[:, 7:8]
                aC = ccol[:, 8:9]    # alpha
                naC = ccol[:, 9:10]  # -alpha
                kaC = ccol[:, 10:11]  # +alpha*KAP
                kbC = ccol[:, 11:12]  # -alpha*KAP
                rC = ccol[:, 12:13]
                RZN = ccol[:, 13:14]
                bC = ccol[:, 14:15]  # beta

                for it in range(NIT):
                    # pAp = ||p||^2 (exact: Re<p, (I + i k H) p> = <p,p>)
                    nc.vector.scalar_tensor_tensor(
                        out=scr1[:], in0=pr_[:], scalar=1.0, in1=pr_[:],
                        op0=OP.mult, op1=OP.mult, accum_out=cA)
                    nc.vector.scalar_tensor_tensor(
                        out=scr2[:], in0=pi_[:], scalar=1.0, in1=pi_[:],
                        op0=OP.mult, op1=OP.mult, accum_out=cB)
                    nc.vector.tensor_tensor(out=cPA, in0=cA, in1=cB, op=OP.add)
                    # a = rz / (pAp + 1e-30)
                    nc.vector.tensor_scalar(out=cAcc, in0=cPA, scalar1=1e-30,
                                            scalar2=None, op0=OP.add)
                    nc.vector.reciprocal(out=cAcc, in_=cAcc)
                    nc.vector.tensor_tensor(out=aC, in0=RZ, in1=cAcc, op=OP.mult)
                    nc.vector.tensor_scalar(out=naC, in0=aC, scalar1=-1.0,
                                            scalar2=None, op0=OP.mult)
                    nc.vector.tensor_scalar(out=kaC, in0=aC, scalar1=float(KAP),
                                            scalar2=None, op0=OP.mult)
                    nc.vector.tensor_scalar(out=kbC, in0=kaC, scalar1=-1.0,
                                            scalar2=None, op0=OP.mult)
                    # t = lam .* p (other plane)
                    nc.gpsimd.tensor_tensor(out=tl1[:], in0=lam128[:], in1=pi_[:],
                                            op=OP.mult)
                    nc.gpsimd.tensor_tensor(out=tl2[:], in0=lam128[:], in1=pr_[:],
                                            op=OP.mult)
                    # x += a*p
                    nc.vector.scalar_tensor_tensor(
                        out=xr[:], in0=pr_[:], scalar=aC, in1=xr[:],
                        op0=OP.mult, op1=OP.add)
                    nc.vector.scalar_tensor_tensor(
                        out=xi[:], in0=pi_[:], scalar=aC, in1=xi[:],
                        op0=OP.mult, op1=OP.add)
                    # r -= a*Ap:  rr += -a*pr + a*KAP*t1 ; ri += -a*pi - a*KAP*t2
                    nc.vector.scalar_tensor_tensor(
                        out=rr_[:], in0=pr_[:], scalar=naC, in1=rr_[:],
                        op0=OP.mult, op1=OP.add)
                    nc.vector.scalar_tensor_tensor(
                        out=rr_[:], in0=tl1[:], scalar=kaC, in1=rr_[:],
                        op0=OP.mult, op1=OP.add)
                    nc.vector.scalar_tensor_tensor(
                        out=ri_[:], in0=pi_[:], scalar=naC, in1=ri_[:],
                        op0=OP.mult, op1=OP.add)
                    nc.vector.scalar_tensor_tensor(
                        out=ri_[:], in0=tl2[:], scalar=kbC, in1=ri_[:],
                        op0=OP.mult, op1=OP.add)
                    # rz_new = inv_s2 * ||r||^2
                    nc.vector.scalar_tensor_tensor(
                        out=scr1[:], in0=rr_[:], scalar=1.0, in1=rr_[:],
                        op0=OP.mult, op1=OP.mult, accum_out=cA)
                    nc.vector.scalar_tensor_tensor(
                        out=scr2[:], in0=ri_[:], scalar=1.0, in1=ri_[:],
                        op0=OP.mult, op1=OP.mult, accum_out=cB)
                    nc.vector.tensor_tensor(out=cAcc, in0=cA, in1=cB, op=OP.add)
                    nc.vector.tensor_scalar(out=RZN, in0=cAcc, scalar1=inv_s2,
                                            scalar2=None, op0=OP.mult)
                    # beta = rz_new / (rz + 1e-30)
                    nc.vector.tensor_scalar(out=rC, in0=RZ, scalar1=1e-30,
                                            scalar2=None, op0=OP.add)
                    nc.vector.reciprocal(out=rC, in_=rC)
                    nc.vector.tensor_tensor(out=bC, in0=RZN, in1=rC, op=OP.mult)
                    nc.vector.tensor_copy(RZ, RZN)
                    # p = M^-1 r + beta*p  (z = inv_s2*[(rr - dk*ri), (dk*rr + ri)])
                    nc.scalar.activation(out=pr_[:], in_=pr_[:], func=AF.Copy,
                                         bias=0.0, scale=bC)
                    nc.vector.scalar_tensor_tensor(
                        out=pr_[:], in0=rr_[:], scalar=inv_s2, in1=pr_[:],
                        op0=OP.mult, op1=OP.add)
                    nc.vector.scalar_tensor_tensor(
                        out=pr_[:], in0=ri_[:], scalar=float(-dk * inv_s2), in1=pr_[:],
                        op0=OP.mult, op1=OP.add)
                    nc.scalar.activation(out=pi_[:], in_=pi_[:], func=AF.Copy,
                                         bias=0.0, scale=bC)
                    nc.vector.scalar_tensor_tensor(
                        out=pi_[:], in0=ri_[:], scalar=inv_s2, in1=pi_[:],
                        op0=OP.mult, op1=OP.add)
                    nc.vector.scalar_tensor_tensor(
                        out=pi_[:], in0=rr_[:], scalar=float(dk * inv_s2), in1=pi_[:],
                        op0=OP.mult, op1=OP.add)

                # ---- transpose x into [f, sys] chunks for the inverse DFT ----
                xrT = [rotTp.tile([128, 128], f32, name=f"xrT{k}", tag=f"rrT{k}") for k in range(KCH)]
                xiT = [rotTp.tile([128, 128], f32, name=f"xiT{k}", tag=f"riT{k}") for k in range(KCH)]
                for k in range(KCH):
                    pt = pst.tile([128, 128], f32, tag="pt")
                    nc.tensor.transpose(pt[:], xr[:, k * 128:(k + 1) * 128], ident[:])
                    nc.scalar.copy(xrT[k][:], pt[:])
                    pt2 = pst.tile([128, 128], f32, tag="pt")
                    nc.tensor.transpose(pt2[:], xi[:, k * 128:(k + 1) * 128], ident[:])
                    nc.scalar.copy(xiT[k][:], pt2[:])

                # ---- inverse DFT -> [sys, d], interleave, fp16 out ----
                pxr = psx.tile([128, D], f32, tag="pxr")
                for k in range(KCH):
                    nc.tensor.matmul(pxr[:], xrT[k][:], Fir_s[:, k * D:(k + 1) * D],
                                     start=(k == 0), stop=False)
                for k in range(KCH):
                    nc.tensor.matmul(pxr[:], xiT[k][:], Fin_s[:, k * D:(k + 1) * D],
                                     start=False, stop=(k == KCH - 1))
                pxi = psx.tile([128, D], f32, tag="pxi")
                for k in range(KCH):
                    nc.tensor.matmul(pxi[:], xrT[k][:], Fii_s[:, k * D:(k + 1) * D],
                                     start=(k == 0), stop=False)
                for k in range(KCH):
                    nc.tensor.matmul(pxi[:], xiT[k][:], Fir_s[:, k * D:(k + 1) * D],
                                     start=False, stop=(k == KCH - 1))
                ot = outp.tile([128, 2 * D], f16, tag="ot")
                ov = ot[:].rearrange("p (d t) -> p d t", t=2)
                nc.scalar.copy(ov[:, :, 0], pxr[:])
                nc.vector.tensor_copy(ov[:, :, 1], pxi[:])
                nc.sync.dma_start(x_d[rows, :], ot[:])
    nc.compile()
    return nc


# ------------------------------------------------------------------ host side
_cache = {}


def _make_exec(nc, replicated=()):
    """Mirror of bass2jax.run_bass_via_pjrt's multi-core path, but returning
    the jitted callable so constant inputs can stay device-resident between
    calls.  Inputs/outputs are GLOBAL arrays (axis 0 = concat of shards)."""
    import jax
    from jax.sharding import Mesh, PartitionSpec
    from jax.experimental.shard_map import shard_map
    from concourse import bass2jax, mybir as _mb

    bass2jax.install_neuronx_cc_hook()
    partition_name = (nc.partition_id_tensor.name
                      if nc.partition_id_tensor else None)
    in_names, out_names, out_avals, zero_outs = [], [], [], []
    for alloc in nc.m.functions[0].allocations:
        if not isinstance(alloc, _mb.MemoryLocationSet):
            continue
        name = alloc.memorylocations[0].name
        if alloc.kind == "ExternalInput":
            if name != partition_name:
                in_names.append(name)
        elif alloc.kind == "ExternalOutput":
            out_names.append(name)
            shape = tuple(alloc.tensor_shape)
            dtype = _mb.dt.np(alloc.dtype)
            out_avals.append(jax.core.ShapedArray(shape, dtype))
            zero_outs.append(((NCORES * shape[0],) + shape[1:], dtype))
    n_params = len(in_names)
    all_in = list(in_names) + list(out_names)
    if partition_name is not None:
        all_in.append(partition_name)

    def _body(*args):
        operands = list(args)
        if partition_name is not None:
            operands.append(bass2jax.partition_id_tensor())
        return tuple(bass2jax._bass_exec_p.bind(
            *operands,
            out_avals=tuple(out_avals),
            in_names=tuple(all_in),
            out_names=tuple(out_names),
            lowering_input_output_aliases=(),
            sim_require_finite=True,
            sim_require_nnan=True,
            nc=nc,
        ))

    devices = jax.devices()[:NCORES]
    mesh = Mesh(np.asarray(devices), ("core",))
    n_outs = len(out_names)
    in_specs = tuple(
        PartitionSpec() if nm in replicated else PartitionSpec("core")
        for nm in in_names
    ) + (PartitionSpec("core"),) * n_outs
    sharded = jax.jit(
        shard_map(_body, mesh=mesh,
                  in_specs=in_specs,
                  out_specs=(PartitionSpec("core"),) * n_outs,
                  check_rep=False),
        donate_argnums=tuple(range(n_params, n_params + n_outs)),
        keep_unused=True,
    )

    def run(feed):  # feed: dict name -> global array (np or jax)
        import jax.numpy as jnp
        args = [feed[n] for n in in_names]
        zs = [jnp.zeros(shp, dt) for shp, dt in zero_outs]
        return sharded(*args, *zs)

    return run, out_names


_CONST_NAMES = ("A1", "A2", "A3", "Fir", "Fii", "Fin", "aabs", "lam")


def _get_state(alpha, edge_weights):
    """Constants (host + device-resident) and compiled executor, memoized."""
    key = (np.asarray(edge_weights, np.float64).tobytes(),
           np.asarray(alpha, np.float64).tobytes())
    st = _cache.get("state")
    if st is not None and st["key"] == key:
        return st
    c = _host_matrices(np.asarray(edge_weights, np.float64),
                       np.asarray(alpha, np.float64))
    nc = _build_kernel(c["deg"])
    run, out_names = _make_exec(nc, replicated=_CONST_NAMES)
    st = dict(key=key, c=c, nc=nc, run=run, out_names=out_names, dev_consts=None)
    _cache["state"] = st
    return st


def kernel(psi_r, psi_i, alpha, edge_weights):
    st = _get_state(alpha, edge_weights)
    c = st["c"]
    pr16 = np.asarray(psi_r, np.float32).reshape(N, D).astype(np.float16)
    pi16 = np.asarray(psi_i, np.float32).reshape(N, D).astype(np.float16)
    try:
        return _run_chained(st, pr16, pi16)
    except Exception:
        return _run_safe(st, pr16, pi16)


def _run_chained(st, pr16, pi16):
    import jax
    from jax.sharding import Mesh, PartitionSpec, NamedSharding

    if st["dev_consts"] is None:
        devices = jax.devices()[:NCORES]
        mesh = Mesh(np.asarray(devices), ("core",))
        repl = NamedSharding(mesh, PartitionSpec())
        st["dev_consts"] = {nm: jax.device_put(st["c"][nm], repl)
                            for nm in _CONST_NAMES}
        jax.block_until_ready(list(st["dev_consts"].values()))
    feed = dict(pr=pr16, pi=pi16, **st["dev_consts"])
    out = st["run"](feed)
    xo = dict(zip(st["out_names"], out))["xout"]
    xo.copy_to_host_async()
    x = np.asarray(xo)                                   # [N, 2D] fp16
    return x.astype(np.float32).reshape(B, S, D, 2)


def _run_safe(st, pr16, pi16):
    c = st["c"]
    core_ids = list(range(NCORES))
    feeds = []
    for ci in core_ids:
        rows = slice(ci * NSYS, (ci + 1) * NSYS)
        fd = dict(pr=pr16[rows], pi=pi16[rows])
        for nm in _CONST_NAMES:
            fd[nm] = c[nm]
        feeds.append(fd)
    res = run_bass_kernel_spmd(st["nc"], feeds, core_ids)
    x = np.concatenate([res.results[ci]["xout"] for ci in core_ids], axis=0)
    return x.astype(np.float32).reshape(B, S, D, 2)
